# revision 1
# baseline (speedup 1.0000x reference)
"""TRN2 Bass kernel for nn_ChartOperator (sparse_attention).

Math (B=4, N=4096, PD=1024, D=16, S=64, ALL=1024):
  P = x @ W_r + b_r
  L = P[..., :ALL].reshape(n, D, S); R = P[..., ALL:].reshape(n, D, S)
  w = softmax_a(L)
  Q[n, d, s] = sum_{a<d} w[n,a,s] * R[n+a+1, d-1-a, s]
  (last D rows of each batch: Q[t+d>=16] zeroed)
  out = Q.reshape(n, ALL) @ W_w + b_w

Sharding: 8 cores, data-parallel over flattened (B*N) rows, 2048 rows/core
plus a 128-row forward halo (zero-padded at batch boundaries; the affected
outputs are exactly the masked ones).

Per-core pipeline (bf16 matmuls, fp32 PSUM):
  1. Reader computes P TRANSPOSED: psum[(d2,s64), n512] = W_r_slab.T @ xT
     (bias via K=1 matmul with b_r slab as stationary, ones moving).
  2. ACT exp/copy writes the banded-MAC layout directly:
     e chain  et_all[(g2,s64), a16, n1024]   (g: row-blocks 0-7 / 8-15)
     r chain  rt_all[(g2,s64), c16, n1152]   (blocks 0-8 / 8-16 incl halo)
  3. DVE: Z = reduce over a; reciprocal; normalize et_all in place.
  4. DVE banded products p[s, c, n] = w[s,a,n]*r[s,c,n+a+1] (single op per a)
  5. PE identity-matmuls accumulate products into PSUM Q[(g,s), d, n]
     with shrinking d-windows (d = a+c+1)
  6. ACT strided copies Q -> qt[(dsub2,s64), k8, n128] bf16 (writer lhsT)
  7. writer matmuls out[n128, 1024] = qt.T @ W_w + b_w -> DMA out
"""
import numpy as np
import ml_dtypes
from contextlib import ExitStack

import concourse.bass as bass
import concourse.tile as tile
from concourse import bacc, mybir
from concourse import bass_utils

BF16 = mybir.dt.bfloat16
F32 = mybir.dt.float32
bfnp = ml_dtypes.bfloat16

B, N, PD = 4, 4096, 1024
D, S = 16, 64
ALL = D * S
ROWS_PER_CORE = 2048
NROWS = 2176                   # + 128-row halo
NCP = 8

_cache = {}


def _build(debug=False):
    nc = bacc.Bacc("TRN2", target_bir_lowering=False, debug=False, num_devices=8)

    xT_d = nc.dram_tensor("xT", [8, 128, NROWS], BF16, kind="ExternalInput").ap()
    wr_d = nc.dram_tensor("wr", [8, 128, 2048], BF16, kind="ExternalInput").ap()
    ww_d = nc.dram_tensor("ww", [8, 128, 1024], BF16, kind="ExternalInput").ap()
    br_d = nc.dram_tensor("br", [128, 16], F32, kind="ExternalInput").ap()
    ident_d = nc.dram_tensor("ident", [128, 128], BF16, kind="ExternalInput").ap()
    qmask_d = nc.dram_tensor("qmask", [128, 8, 128], BF16, kind="ExternalInput").ap()
    out_d = nc.dram_tensor("out", [16, 128, 1024], F32, kind="ExternalOutput").ap()

    with tile.TileContext(nc) as tc, ExitStack() as ctx:
        cpool = ctx.enter_context(tc.tile_pool(name="cpool", bufs=1))
        ps512 = ctx.enter_context(tc.tile_pool(name="ps512", bufs=2, space="PSUM"))
        wps = ctx.enter_context(tc.tile_pool(name="wps", bufs=2, space="PSUM"))
        ztp = ctx.enter_context(tc.tile_pool(name="ztp", bufs=1))
        rzp = ctx.enter_context(tc.tile_pool(name="rzp", bufs=1))
        prodp = ctx.enter_context(tc.tile_pool(name="prodp", bufs=4))
        macp = ctx.enter_context(tc.tile_pool(name="macp", bufs=1, space="PSUM"))
        qtp = ctx.enter_context(tc.tile_pool(name="qtp", bufs=3))
        osbp = ctx.enter_context(tc.tile_pool(name="osbp", bufs=2))

        # --- persistent constants / big buffers
        xk = [cpool.tile([128, NROWS], BF16, name=f"xk{i}", tag=f"xk{i}")
              for i in range(8)]
        for ks in range(8):
            nc.gpsimd.dma_start(xk[ks][:], xT_d[ks])
        wr_sb = cpool.tile([128, 8, 2048], BF16)
        nc.gpsimd.dma_start(wr_sb[:], wr_d[:].rearrange("k p c -> p k c"))
        ww_sb = cpool.tile([128, 8, 1024], BF16)
        nc.gpsimd.dma_start(ww_sb[:], ww_d[:].rearrange("k p c -> p k c"))
        br_sb = cpool.tile([128, 16], F32)
        nc.gpsimd.dma_start(br_sb[:], br_d[:])
        ident = cpool.tile([128, 128], BF16)
        nc.gpsimd.dma_start(ident[:], ident_d[:])
        qmask = cpool.tile([128, 8, 128], BF16)
        nc.gpsimd.dma_start(qmask[:], qmask_d[:])
        et_all = cpool.tile([128, 16, 1024], BF16)   # [(g2,s64), a, n-chain]
        rt_all = cpool.tile([128, 16, 1152], BF16)   # [(g2,s64), c, n-chain]

        # ---------------- Loop 1: transposed reader + layout writes
        # jj: 4 supertiles of 512 rows + 1 halo tile of 128 rows
        for jj in (0, 2, 1, 3, 4):
            nwin = 128 if jj == 4 else 512
            n0 = jj * 512
            g = 0 if jj < 2 else 1
            for u in range(16):               # col slabs: 0-7 = L, 8-15 = R
                is_l = u < 8
                if is_l and jj == 4:
                    continue                  # halo rows: R only
                ps = ps512.tile([128, 512], F32, tag="ps512", name="ps")
                for ks in range(8):
                    nc.tensor.matmul(ps[:, :nwin], wr_sb[:, ks, 128 * u:128 * (u + 1)],
                                     xk[ks][:, n0:n0 + nwin],
                                     start=(ks == 0), stop=(ks == 7))
                for dsub in range(2):
                    src = ps[64 * dsub:64 * dsub + 64, :nwin]
                    bias = br_sb[64 * dsub:64 * dsub + 64, u:u + 1]
                    AF = mybir.ActivationFunctionType
                    if is_l:
                        a = 2 * u + dsub
                        dst = et_all[64 * g:64 * g + 64, a,
                                     n0 - 1024 * g:n0 - 1024 * g + nwin]
                        nc.scalar.activation(dst, src, AF.Exp, bias=bias)
                    else:
                        c = 2 * (u - 8) + dsub
                        # g0 chain: blocks 0..8 ; g1 chain: blocks 8..16
                        if jj < 2:
                            nc.scalar.activation(rt_all[0:64, c, n0:n0 + 512], src,
                                                 AF.Identity, bias=bias)
                        elif jj == 2:   # blocks 8-11: both chains
                            nc.scalar.activation(rt_all[0:64, c, 1024:1152],
                                                 ps[64 * dsub:64 * dsub + 64, 0:128],
                                                 AF.Identity, bias=bias)
                            nc.scalar.activation(rt_all[64:128, c, 0:512], src,
                                                 AF.Identity, bias=bias)
                        elif jj == 3:
                            nc.scalar.activation(rt_all[64:128, c, 512:1024], src,
                                                 AF.Identity, bias=bias)
                        else:           # halo block 16
                            nc.scalar.activation(rt_all[64:128, c, 1024:1152], src,
                                                 AF.Identity, bias=bias)

            if jj in (2, 3):
                # normalize chain-window: w0 = supertiles {0,2}, w1 = {1,3}
                # (loop order 0,2,1,3 makes w0 ready after the 2nd supertile)
                lo = (jj - 2) * 512
                zt = ztp.tile([128, 512], F32, tag="ztp", name="zt")
                e_na = et_all[:, :, lo:lo + 512].rearrange("p a n -> p n a")
                nc.vector.tensor_reduce(zt[:], e_na, axis=mybir.AxisListType.X,
                                        op=mybir.AluOpType.add)
                rz = rzp.tile([128, 512], F32, tag="rzp", name="rz")
                nc.vector.reciprocal(rz[:], zt[:])
                rz3 = rz[:].rearrange("p (o n) -> p o n", o=1).to_broadcast((128, 16, 512))
                ew = et_all[:, :, lo:lo + 512]
                nc.vector.tensor_mul(ew, ew, rz3)

        # ---------------- Loop 2+3: banded MAC + writer per chunklet-pair
        for cp in range(NCP):
            mp = macp.tile([128, 16, 128], F32, tag="macp", name="mp")
            nc.vector.memset(mp[:, 0, :], 0.0)
            n0 = 128 * cp
            for a in range(15):
                cnt = 15 - a
                p = prodp.tile([128, 15, 128], BF16, tag="prodp", name="p")
                eb = et_all[:, a:a + 1, n0:n0 + 128].to_broadcast((128, cnt, 128))
                # odd (a+1)-shifts run at DVE 1x (4B misalignment); split
                # those across DVE and GpSimd to balance the two engines
                if a % 2 == 0 and cnt >= 2:
                    cs = max(1, (3 * cnt) // 5)
                    nc.vector.tensor_mul(p[:, 0:cs, :], eb[:, 0:cs, :],
                                         rt_all[:, 0:cs, n0 + a + 1:n0 + a + 129])
                    nc.gpsimd.tensor_mul(p[:, cs:cnt, :], eb[:, 0:cnt - cs, :],
                                         rt_all[:, cs:cnt, n0 + a + 1:n0 + a + 129])
                else:
                    nc.vector.tensor_mul(p[:, 0:cnt, :], eb,
                                         rt_all[:, 0:cnt, n0 + a + 1:n0 + a + 129])
                for b in range(4):
                    d_lo = max(a + 1, 4 * b)
                    d_hi = 4 * b + 4
                    if d_lo >= d_hi:
                        continue
                    last_a = min(14, 4 * b + 2)
                    nc.tensor.matmul(mp[:, d_lo:d_hi, :], ident[:],
                                     p[:, d_lo - a - 1:d_hi - a - 1, :],
                                     start=(a == 0), stop=(a == last_a))

            for g in range(2):
                cb = 8 * g + cp
                qt = qtp.tile([128, 8, 128], BF16, tag="qtp", name="qt")
                for dsub in range(2):
                    csrc = mp[64 * g:64 * g + 64, dsub::2, :]
                    cdst = qt[64 * dsub:64 * dsub + 64, :, :]
                    if g == 0:
                        nc.scalar.copy(cdst, csrc)
                    else:
                        nc.vector.tensor_copy(cdst, csrc)
                if cb == 15:
                    nc.vector.tensor_mul(qt[:], qt[:], qmask[:])
                osb = osbp.tile([128, 1024], F32, tag="osbp", name="osb")
                for h in range(2):
                    wp = wps.tile([128, 512], F32, tag="wps", name="wp")
                    for k in range(8):
                        nc.tensor.matmul(wp[:], qt[:, k, :],
                                         ww_sb[:, k, h * 512:(h + 1) * 512],
                                         start=(k == 0), stop=(k == 7))
                    nc.vector.tensor_copy(osb[:, h * 512:(h + 1) * 512], wp[:])
                nc.gpsimd.dma_start(out_d[cb], osb[:])

    nc.compile()
    return nc


def _host_prep(x, W_r, b_r, W_w, b_w):
    """Build the 8 per-core input maps."""
    xf = np.asarray(x, np.float32).reshape(B * N, PD)
    wr = np.asarray(W_r, np.float32).astype(bfnp)
    ww = np.asarray(W_w, np.float32).astype(bfnp)
    br = np.ascontiguousarray(
        np.asarray(b_r, np.float32).reshape(16, 128).T)
    wr_t = np.ascontiguousarray(wr.reshape(8, 128, 2048))
    ww_t = np.ascontiguousarray(ww.reshape(8, 128, 1024))
    ident = np.eye(128, dtype=np.float32).astype(bfnp)

    in_maps = []
    for c in range(8):
        lo = c * ROWS_PER_CORE
        chunk = np.zeros((NROWS, PD), np.float32)
        chunk[:ROWS_PER_CORE] = xf[lo:lo + ROWS_PER_CORE]
        if c % 2 == 0:
            chunk[ROWS_PER_CORE:] = xf[lo + ROWS_PER_CORE: lo + NROWS]
        # xT[ks, k, n] = chunk[n, 128*ks + k]
        xt = np.ascontiguousarray(
            chunk.astype(bfnp).reshape(NROWS, 8, 128).transpose(1, 2, 0))
        qmask = np.ones((128, 8, 128), np.float32)
        if c % 2 == 1:
            dsub = (np.arange(128)[:, None, None] // 64)
            k = np.arange(8)[None, :, None]
            n = np.arange(128)[None, None, :]
            bad = (n >= 112) & ((n - 112 + 2 * k + dsub) >= 16)
            qmask[np.broadcast_to(bad, (128, 8, 128))] = 0.0
        in_maps.append({
            "xT": xt,
            "wr": wr_t, "ww": ww_t, "br": br,
            "ident": ident, "qmask": qmask.astype(bfnp),
        })
    return in_maps


def kernel(x, W_r, b_r, W_w, b_w):
    if "nc" not in _cache:
        _cache["nc"] = _build()
    nc = _cache["nc"]
    in_maps = _host_prep(x, W_r, b_r, W_w, b_w)
    res = bass_utils.run_bass_kernel_spmd(nc, in_maps, core_ids=list(range(8)))
    out = np.concatenate([r["out"].reshape(ROWS_PER_CORE, ALL)
                          for r in res.results], axis=0)
    out = out.reshape(B, N, ALL).astype(np.float32)
    out += np.asarray(b_w, np.float32).reshape(1, 1, ALL)
    return np.ascontiguousarray(out)



# revision 5
# speedup vs baseline: 1.0494x; 1.0494x over previous
"""TRN2 Bass kernel for nn_ChartOperator (sparse_attention).

Math (B=4, N=4096, PD=1024, D=16, S=64, ALL=1024):
  P = x @ W_r + b_r
  L = P[..., :ALL].reshape(n, D, S); R = P[..., ALL:].reshape(n, D, S)
  w = softmax_a(L)
  Q[n, d, s] = sum_{a<d} w[n,a,s] * R[n+a+1, d-1-a, s]
  (last D rows of each batch: Q[t+d>=16] zeroed)
  out = Q.reshape(n, ALL) @ W_w + b_w

Sharding: 8 cores, data-parallel over flattened (B*N) rows, 2048 rows/core
plus a 16-row forward halo (zero-padded at batch boundaries; the affected
outputs are exactly the masked ones).

Per-core pipeline (bf16 matmuls, fp32 PSUM), software-pipelined so the
banded-MAC + writer chunklets interleave with the reader supertiles:
  PE order: rd(0) rd(2) rd(1) mac(0..2) rd(3,R-first) rd(halo) mac(3..7)
  1. Reader computes P TRANSPOSED: psum[(d2,s64), n512] = W_r_slab.T @ xT.
  2. ACT exp/copy writes banded-MAC layout:
     e chain  et_all[(g2,s64), a16, n1024]  (g: row-blocks 0-7 / 8-15)
     r chain  rt_all[(g2,s64), c16, n1040]  (blocks 0-8 / 8-16 incl halo)
     rt1_all = rt_all shifted by 1 col (so every banded product has a
     4B-aligned operand -> DVE 2x mode).
  3. softmax denom Z on GpSimd (strided reduce); DVE recip + normalize.
  4. banded products p[s, c, n] = w[s,a,n]*r[s,c,n+a+1]: even-a via rt1
     (DVE 2x), a in {0,2,4} offloaded to GpSimd.
  5. PE identity-matmuls accumulate products into PSUM Q[(g,s), d, n]
     with shrinking d-windows (d = a+c+1)
  6. ACT/DVE strided copies Q -> qt[(dsub2,s64), k8, n128] bf16
  7. writer matmuls out[n128, 1024] = qt.T @ W_w + b_w -> DMA out
"""
import numpy as np
import ml_dtypes
from contextlib import ExitStack

import concourse.bass as bass
import concourse.tile as tile
from concourse import bacc, mybir
from concourse import bass_utils

BF16 = mybir.dt.bfloat16
F32 = mybir.dt.float32
bfnp = ml_dtypes.bfloat16

B, N, PD = 4, 4096, 1024
D, S = 16, 64
ALL = D * S
ROWS_PER_CORE = 2048
HALO = 16
NROWS = ROWS_PER_CORE + HALO   # 2064
RTC = 1040                     # rt chain length (needs cols <= 1039)
NCP = 8
GP_A = (0, 2, 4)               # product shifts computed on GpSimd

_cache = {}


def _build(debug=False):
    nc = bacc.Bacc("TRN2", target_bir_lowering=False, debug=False, num_devices=8)

    xT_d = nc.dram_tensor("xT", [8, 128, NROWS], BF16, kind="ExternalInput").ap()
    wr_d = nc.dram_tensor("wr", [16, 128, 1024], BF16, kind="ExternalInput").ap()
    ww_d = nc.dram_tensor("ww", [8, 128, 1024], BF16, kind="ExternalInput").ap()
    br_d = nc.dram_tensor("br", [128, 16], F32, kind="ExternalInput").ap()
    ident_d = nc.dram_tensor("ident", [128, 128], BF16, kind="ExternalInput").ap()
    qmask_d = nc.dram_tensor("qmask", [128, 8, 128], BF16, kind="ExternalInput").ap()
    out_d = nc.dram_tensor("out", [16, 128, 1024], F32, kind="ExternalOutput").ap()

    AF = mybir.ActivationFunctionType

    with tile.TileContext(nc) as tc, ExitStack() as ctx:
        cpool = ctx.enter_context(tc.tile_pool(name="cpool", bufs=1))
        ps512 = ctx.enter_context(tc.tile_pool(name="ps512", bufs=2, space="PSUM"))
        wps = ctx.enter_context(tc.tile_pool(name="wps", bufs=2, space="PSUM"))
        macp = ctx.enter_context(tc.tile_pool(name="macp", bufs=1, space="PSUM"))
        prodp = ctx.enter_context(tc.tile_pool(name="prodp", bufs=3))
        qtp = ctx.enter_context(tc.tile_pool(name="qtp", bufs=2))
        osbp = ctx.enter_context(tc.tile_pool(name="osbp", bufs=2))

        # --- persistent tiles
        xkj = [cpool.tile([128, 8, 512], BF16, name=f"xkj{j}", tag=f"xkj{j}")
               for j in range(4)]
        xh = cpool.tile([128, 8, HALO], BF16, name="xh", tag="xh")
        wr_sb = cpool.tile([128, 16, 1024], BF16)   # (p, u, ks*128+c)
        ww_sb = cpool.tile([128, 8, 1024], BF16)
        br_sb = cpool.tile([128, 16], F32)
        ident = cpool.tile([128, 128], BF16)
        qmask = cpool.tile([128, 8, 128], BF16)
        et_all = cpool.tile([128, 16, 1024], BF16)   # [(g2,s64), a, n-chain]
        rt_all = cpool.tile([128, 16, RTC], BF16)    # [(g2,s64), c, n-chain]
        rt1_all = cpool.tile([128, 15, RTC], BF16)   # rt shifted by +1 col
        zts = cpool.tile([128, 8, 128], F32)         # denom tree scratch
        zt = cpool.tile([128, 128], F32)             # softmax denom
        rzb = cpool.tile([128, 128], BF16)           # 1/Z bf16

        # --- DMAs on sync queue, ordered by first use
        nc.sync.dma_start(xkj[0][:], xT_d[:, :, 0:512].rearrange("k p n -> p k n"))
        nc.sync.dma_start(wr_sb[:, 0, :], wr_d[0])
        nc.sync.dma_start(xkj[2][:], xT_d[:, :, 1024:1536].rearrange("k p n -> p k n"))
        nc.sync.dma_start(wr_sb[:, 1:8, :], wr_d[1:8].rearrange("u p c -> p u c"))
        nc.sync.dma_start(wr_sb[:, 8:16, :], wr_d[8:16].rearrange("u p c -> p u c"))
        nc.sync.dma_start(br_sb[:], br_d[:])
        nc.sync.dma_start(xkj[1][:], xT_d[:, :, 512:1024].rearrange("k p n -> p k n"))
        nc.sync.dma_start(ident[:], ident_d[:])
        nc.sync.dma_start(xkj[3][:], xT_d[:, :, 1536:2048].rearrange("k p n -> p k n"))
        nc.sync.dma_start(xh[:], xT_d[:, :, 2048:NROWS].rearrange("k p n -> p k n"))
        nc.sync.dma_start(ww_sb[:, 0:4, :], ww_d[0:4].rearrange("k p c -> p k c"))
        nc.sync.dma_start(ww_sb[:, 4:8, :], ww_d[4:8].rearrange("k p c -> p k c"))
        nc.sync.dma_start(qmask[:], qmask_d[:])

        def reader(jj, uorder=None):
            """One 512-row supertile (jj<4) or the 16-row halo (jj==4)."""
            halo = jj == 4
            nwin = HALO if halo else 512
            n0 = jj * 512
            g = 0 if jj < 2 else 1
            xt = xh if halo else xkj[jj]
            for u in (uorder or range(16)):
                is_l = u < 8
                if is_l and halo:
                    continue                  # halo rows: R only
                ps = ps512.tile([128, 512], F32, tag="ps512", name="ps")
                for ks in range(8):
                    nc.tensor.matmul(ps[:, :nwin], wr_sb[:, u, 128 * ks:128 * ks + 128],
                                     xt[:, ks, :nwin],
                                     start=(ks == 0), stop=(ks == 7))
                for dsub in range(2):
                    src = ps[64 * dsub:64 * dsub + 64, :nwin]
                    bias = br_sb[64 * dsub:64 * dsub + 64, u:u + 1]
                    if is_l:
                        a = 2 * u + dsub
                        dst = et_all[64 * g:64 * g + 64, a,
                                     n0 - 1024 * g:n0 - 1024 * g + nwin]
                        nc.scalar.activation(dst, src, AF.Exp, bias=bias)
                    else:
                        c = 2 * (u - 8) + dsub
                        # g0 chain: blocks 0..8+ ; g1 chain: blocks 8..16+
                        if jj < 2:
                            nc.scalar.activation(rt_all[0:64, c, n0:n0 + 512], src,
                                                 AF.Identity, bias=bias)
                        elif jj == 2:   # blocks 8-11: both chains
                            nc.scalar.activation(rt_all[0:64, c, 1024:RTC],
                                                 ps[64 * dsub:64 * dsub + 64, 0:RTC - 1024],
                                                 AF.Identity, bias=bias)
                            nc.scalar.activation(rt_all[64:128, c, 0:512], src,
                                                 AF.Identity, bias=bias)
                        elif jj == 3:
                            nc.scalar.activation(rt_all[64:128, c, 512:1024], src,
                                                 AF.Identity, bias=bias)
                        else:           # halo block 16
                            nc.scalar.activation(rt_all[64:128, c, 1024:1024 + HALO],
                                                 src, AF.Identity, bias=bias)

        def rt1_copy(lo, hi):
            # rt1[:, c, j] = rt[:, c, j+1]
            nc.scalar.copy(rt1_all[:, :, lo:hi], rt_all[:, 0:15, lo + 1:hi + 1])

        def norm128(w):
            """softmax denom + normalize et over cols [128w, 128w+128)."""
            lo = 128 * w
            e = et_all[:, :, lo:lo + 128]
            nc.vector.tensor_tensor(zts[:], e[:, 0:8, :], e[:, 8:16, :],
                                    op=mybir.AluOpType.add)
            nc.vector.tensor_tensor(zts[:, 0:4, :], zts[:, 0:4, :], zts[:, 4:8, :],
                                    op=mybir.AluOpType.add)
            nc.vector.tensor_tensor(zts[:, 0:2, :], zts[:, 0:2, :], zts[:, 2:4, :],
                                    op=mybir.AluOpType.add)
            nc.vector.tensor_tensor(zt[:], zts[:, 0, :], zts[:, 1, :],
                                    op=mybir.AluOpType.add)
            nc.vector.reciprocal(zt[:], zt[:])
            nc.vector.tensor_copy(rzb[:], zt[:])
            rz3 = rzb[:].rearrange("p (o n) -> p o n", o=1) \
                .to_broadcast((128, 15, 128))
            ew = et_all[:, 0:15, lo:lo + 128]
            nc.vector.tensor_mul(ew, ew, rz3)

        def mac(cp):
            """Banded MAC + writer for one 128-row chunklet pair."""
            mp = macp.tile([128, 16, 128], F32, tag="macp", name="mp")
            nc.vector.memset(mp[:, 0, :], 0.0)
            n0 = 128 * cp
            for a in range(15):
                cnt = 15 - a
                p = prodp.tile([128, 15, 128], BF16, tag="prodp", name="p")
                eb = et_all[:, a:a + 1, n0:n0 + 128].to_broadcast((128, cnt, 128))
                eng = nc.gpsimd if a in GP_A else nc.vector
                if a % 2 == 0:
                    # even shift a+1 is 2B-misaligned on rt; use the
                    # pre-shifted rt1 so DVE runs in 2x mode
                    eng.tensor_mul(p[:, 0:cnt, :], eb,
                                   rt1_all[:, 0:cnt, n0 + a:n0 + a + 128])
                else:
                    eng.tensor_mul(p[:, 0:cnt, :], eb,
                                   rt_all[:, 0:cnt, n0 + a + 1:n0 + a + 129])
                for b in range(4):
                    d_lo = max(a + 1, 4 * b)
                    d_hi = 4 * b + 4
                    if d_lo >= d_hi:
                        continue
                    last_a = min(14, 4 * b + 2)
                    nc.tensor.matmul(mp[:, d_lo:d_hi, :], ident[:],
                                     p[:, d_lo - a - 1:d_hi - a - 1, :],
                                     start=(a == 0), stop=(a == last_a))

            for g in range(2):
                cb = 8 * g + cp
                qt = qtp.tile([128, 8, 128], BF16, tag="qtp", name="qt")
                for dsub in range(2):
                    csrc = mp[64 * g:64 * g + 64, dsub::2, :]
                    cdst = qt[64 * dsub:64 * dsub + 64, :, :]
                    if g == 0:
                        nc.scalar.copy(cdst, csrc)
                    else:
                        nc.vector.tensor_copy(cdst, csrc)
                if cb == 15:
                    nc.vector.tensor_mul(qt[:], qt[:], qmask[:])
                osb = osbp.tile([128, 1024], F32, tag="osbp", name="osb")
                for h in range(2):
                    wp = wps.tile([128, 512], F32, tag="wps", name="wp")
                    for k in range(8):
                        nc.tensor.matmul(wp[:], qt[:, k, :],
                                         ww_sb[:, k, h * 512:(h + 1) * 512],
                                         start=(k == 0), stop=(k == 7))
                    nc.vector.tensor_copy(osb[:, h * 512:(h + 1) * 512], wp[:])
                nc.sync.dma_start(out_d[cb], osb[:])

        # ---------------- interleaved schedule
        reader(0)
        reader(2)
        rt1_copy(0, 511)
        for w in range(4):
            norm128(w)
        reader(1)
        mac(0)
        mac(1)
        mac(2)
        # last supertile R-slabs first so rt (and rt1) unblock mac(3) early
        reader(3, uorder=list(range(8, 16)))
        rt1_copy(511, 655)
        reader(3, uorder=list(range(0, 8)))
        reader(4)
        rt1_copy(655, 1023)
        rt1_copy(1023, RTC - 1)
        mac(3)
        norm128(4)
        mac(4)
        norm128(5)
        mac(5)
        norm128(6)
        mac(6)
        norm128(7)
        mac(7)

    nc.compile()
    return nc


def _host_prep(x, W_r, b_r, W_w, b_w):
    """Build the 8 per-core input maps."""
    xf = np.asarray(x, np.float32).reshape(B * N, PD)
    wr = np.asarray(W_r, np.float32).astype(bfnp)
    ww = np.asarray(W_w, np.float32).astype(bfnp)
    br = np.ascontiguousarray(
        np.asarray(b_r, np.float32).reshape(16, 128).T)
    # wr_d[u, p, ks*128+c] = W_r[128*ks + p, 128*u + c]
    wr_t = np.ascontiguousarray(
        wr.reshape(8, 128, 16, 128).transpose(2, 1, 0, 3).reshape(16, 128, 1024))
    ww_t = np.ascontiguousarray(ww.reshape(8, 128, 1024))
    ident = np.eye(128, dtype=np.float32).astype(bfnp)

    in_maps = []
    for c in range(8):
        lo = c * ROWS_PER_CORE
        chunk = np.zeros((NROWS, PD), np.float32)
        chunk[:ROWS_PER_CORE] = xf[lo:lo + ROWS_PER_CORE]
        if c % 2 == 0:
            chunk[ROWS_PER_CORE:] = xf[lo + ROWS_PER_CORE: lo + NROWS]
        # xT[ks, k, n] = chunk[n, 128*ks + k]
        xt = np.ascontiguousarray(
            chunk.astype(bfnp).reshape(NROWS, 8, 128).transpose(1, 2, 0))
        qmask = np.ones((128, 8, 128), np.float32)
        if c % 2 == 1:
            dsub = (np.arange(128)[:, None, None] // 64)
            k = np.arange(8)[None, :, None]
            n = np.arange(128)[None, None, :]
            bad = (n >= 112) & ((n - 112 + 2 * k + dsub) >= 16)
            qmask[np.broadcast_to(bad, (128, 8, 128))] = 0.0
        in_maps.append({
            "xT": xt,
            "wr": wr_t, "ww": ww_t, "br": br,
            "ident": ident, "qmask": qmask.astype(bfnp),
        })
    return in_maps


def kernel(x, W_r, b_r, W_w, b_w):
    if "nc" not in _cache:
        _cache["nc"] = _build()
    nc = _cache["nc"]
    in_maps = _host_prep(x, W_r, b_r, W_w, b_w)
    res = bass_utils.run_bass_kernel_spmd(nc, in_maps, core_ids=list(range(8)))
    out = np.concatenate([r["out"].reshape(ROWS_PER_CORE, ALL)
                          for r in res.results], axis=0)
    out = out.reshape(B, N, ALL).astype(np.float32)
    out += np.asarray(b_w, np.float32).reshape(1, 1, ALL)
    return np.ascontiguousarray(out)


# revision 11
# speedup vs baseline: 1.1956x; 1.1393x over previous
"""TRN2 Bass kernel for nn_ChartOperator (sparse_attention).

Math (B=4, N=4096, PD=1024, D=16, S=64, ALL=1024):
  P = x @ W_r + b_r
  L = P[..., :ALL].reshape(n, D, S); R = P[..., ALL:].reshape(n, D, S)
  w = softmax_a(L)
  Q[n, d, s] = sum_{a<d} w[n,a,s] * R[n+a+1, d-1-a, s]
  (last D rows of each batch: Q[t+d>=16] zeroed)
  out = Q.reshape(n, ALL) @ W_w + b_w

Sharding: 8 cores, data-parallel over flattened (B*N) rows, 2048 rows/core
plus a 16-row forward halo (zero-padded at batch boundaries; the affected
outputs are exactly the masked ones).

Per-core pipeline (bf16 matmuls, fp32 PSUM), software-pipelined so the
banded-MAC + writer chunklets interleave with the reader supertiles:
  PE order: rd(0) rd(2) rd(1) mac(0..2) rd(3,R-first) rd(halo) mac(3..7)
  1. Reader computes P TRANSPOSED: psum[(d2,s64), n512] = W_r_slab.T @ xT.
  2. ACT exp/copy writes banded-MAC layout:
     e chain  et_all[(g2,s64), a16, n1024]  (g: row-blocks 0-7 / 8-15)
     r chain  rt_all[(g2,s64), c16, n1040]  (blocks 0-8 / 8-16 incl halo)
     rt1_all = rt_all shifted by 1 col (so every banded product has a
     4B-aligned operand -> DVE 2x mode).
  3. softmax denom Z on GpSimd (strided reduce); DVE recip + normalize.
  4. banded products p[s, c, n] = w[s,a,n]*r[s,c,n+a+1]: even-a via rt1
     (DVE 2x), a in {0,2,4} offloaded to GpSimd.
  5. PE identity-matmuls accumulate products into PSUM Q[(g,s), d, n]
     with shrinking d-windows (d = a+c+1)
  6. ACT/DVE strided copies Q -> qt[(dsub2,s64), k8, n128] bf16
  7. writer matmuls out[n128, 1024] = qt.T @ W_w + b_w -> DMA out
"""
import numpy as np
import ml_dtypes
from contextlib import ExitStack

import concourse.bass as bass
import concourse.tile as tile
from concourse import bacc, mybir
from concourse import bass_utils

BF16 = mybir.dt.bfloat16
F32 = mybir.dt.float32
bfnp = ml_dtypes.bfloat16

B, N, PD = 4, 4096, 1024
D, S = 16, 64
ALL = D * S
ROWS_PER_CORE = 2048
HALO = 16
NROWS = ROWS_PER_CORE + HALO   # 2064
RTC = 1040                     # rt chain length (needs cols <= 1039)
NCP = 8
GP_A = (0, 2)                  # product shifts computed on GpSimd

_cache = {}


def _build(debug=False):
    nc = bacc.Bacc("TRN2", target_bir_lowering=False, debug=False, num_devices=8)

    xT_d = nc.dram_tensor("xT", [8, 128, NROWS], BF16, kind="ExternalInput").ap()
    wr_d = nc.dram_tensor("wr", [16, 128, 1024], BF16, kind="ExternalInput").ap()
    ww_d = nc.dram_tensor("ww", [8, 128, 1024], BF16, kind="ExternalInput").ap()
    br_d = nc.dram_tensor("br", [128, 16], F32, kind="ExternalInput").ap()
    ident_d = nc.dram_tensor("ident", [128, 128], BF16, kind="ExternalInput").ap()
    qmask_d = nc.dram_tensor("qmask", [128, 8, 128], BF16, kind="ExternalInput").ap()
    out_d = nc.dram_tensor("out", [16, 128, 1024], F32, kind="ExternalOutput").ap()

    AF = mybir.ActivationFunctionType

    with tile.TileContext(nc) as tc, ExitStack() as ctx:
        cpool = ctx.enter_context(tc.tile_pool(name="cpool", bufs=1))
        ps512 = ctx.enter_context(tc.tile_pool(name="ps512", bufs=2, space="PSUM"))
        wps = ctx.enter_context(tc.tile_pool(name="wps", bufs=2, space="PSUM"))
        macp = ctx.enter_context(tc.tile_pool(name="macp", bufs=1, space="PSUM"))
        prodp = ctx.enter_context(tc.tile_pool(name="prodp", bufs=3))
        qtp = ctx.enter_context(tc.tile_pool(name="qtp", bufs=2))
        osbp = ctx.enter_context(tc.tile_pool(name="osbp", bufs=2))

        # --- persistent tiles
        xkj = [cpool.tile([128, 8, 512], BF16, name=f"xkj{j}", tag=f"xkj{j}")
               for j in range(4)]
        xh = cpool.tile([128, 8, HALO], BF16, name="xh", tag="xh")
        wr_sb = cpool.tile([128, 16, 1024], BF16)   # (p, u, ks*128+c)
        ww_sb = cpool.tile([128, 8, 1024], BF16)
        br_sb = cpool.tile([128, 16], F32)
        ident = cpool.tile([128, 128], BF16)
        qmask = cpool.tile([128, 8, 128], BF16)
        et_all = cpool.tile([128, 16, 1024], BF16)   # [(g2,s64), a, n-chain]
        rt_all = cpool.tile([128, 16, RTC], BF16)    # [(g2,s64), c, n-chain]
        rt1_all = cpool.tile([128, 15, RTC], BF16)   # rt shifted by +1 col
        zts = cpool.tile([128, 8, 128], F32)         # denom tree scratch
        zt = cpool.tile([128, 128], F32)             # softmax denom
        rzb = cpool.tile([128, 128], BF16)           # 1/Z bf16

        # --- input DMAs, ordered by first use; the two first-mm-critical
        # ones go on the scalar queue so they issue in parallel with sync's
        nc.scalar.dma_start(xkj[0][:], xT_d[:, :, 0:512].rearrange("k p n -> p k n"))
        nc.scalar.dma_start(wr_sb[:, 0, :], wr_d[0])
        nc.sync.dma_start(wr_sb[:, 1:4, :], wr_d[1:4].rearrange("u p c -> p u c"))
        nc.sync.dma_start(wr_sb[:, 4:8, :], wr_d[4:8].rearrange("u p c -> p u c"))
        nc.sync.dma_start(br_sb[:], br_d[:])
        nc.sync.dma_start(wr_sb[:, 8:12, :], wr_d[8:12].rearrange("u p c -> p u c"))
        nc.sync.dma_start(wr_sb[:, 12:16, :], wr_d[12:16].rearrange("u p c -> p u c"))
        nc.sync.dma_start(xkj[2][:], xT_d[:, :, 1024:1536].rearrange("k p n -> p k n"))
        nc.sync.dma_start(xkj[1][:], xT_d[:, :, 512:1024].rearrange("k p n -> p k n"))
        nc.sync.dma_start(ident[:], ident_d[:])
        nc.sync.dma_start(xkj[3][:], xT_d[:, :, 1536:2048].rearrange("k p n -> p k n"))
        nc.sync.dma_start(xh[:], xT_d[:, :, 2048:NROWS].rearrange("k p n -> p k n"))
        nc.sync.dma_start(ww_sb[:, 0:4, :], ww_d[0:4].rearrange("k p c -> p k c"))
        nc.sync.dma_start(ww_sb[:, 4:8, :], ww_d[4:8].rearrange("k p c -> p k c"))
        nc.sync.dma_start(qmask[:], qmask_d[:])

        def reader(jj, uorder=None):
            """One 512-row supertile (jj<4) or the 16-row halo (jj==4)."""
            halo = jj == 4
            nwin = HALO if halo else 512
            n0 = jj * 512
            g = 0 if jj < 2 else 1
            xt = xh if halo else xkj[jj]
            for u in (uorder or range(16)):
                is_l = u < 8
                if is_l and halo:
                    continue                  # halo rows: R only
                ps = ps512.tile([128, 512], F32, tag="ps512", name="ps")
                for ks in range(8):
                    nc.tensor.matmul(ps[:, :nwin], wr_sb[:, u, 128 * ks:128 * ks + 128],
                                     xt[:, ks, :nwin],
                                     start=(ks == 0), stop=(ks == 7))
                for dsub in range(2):
                    src = ps[64 * dsub:64 * dsub + 64, :nwin]
                    bias = br_sb[64 * dsub:64 * dsub + 64, u:u + 1]
                    if is_l:
                        a = 2 * u + dsub
                        dst = et_all[64 * g:64 * g + 64, a,
                                     n0 - 1024 * g:n0 - 1024 * g + nwin]
                        nc.scalar.activation(dst, src, AF.Exp, bias=bias)
                    else:
                        c = 2 * (u - 8) + dsub
                        # g0 chain: blocks 0..8+ ; g1 chain: blocks 8..16+
                        if jj < 2:
                            nc.scalar.activation(rt_all[0:64, c, n0:n0 + 512], src,
                                                 AF.Identity, bias=bias)
                        elif jj == 2:   # blocks 8-11: both chains
                            nc.scalar.activation(rt_all[0:64, c, 1024:RTC],
                                                 ps[64 * dsub:64 * dsub + 64, 0:RTC - 1024],
                                                 AF.Identity, bias=bias)
                            nc.scalar.activation(rt_all[64:128, c, 0:512], src,
                                                 AF.Identity, bias=bias)
                        elif jj == 3:
                            nc.scalar.activation(rt_all[64:128, c, 512:1024], src,
                                                 AF.Identity, bias=bias)
                        else:           # halo block 16
                            nc.scalar.activation(rt_all[64:128, c, 1024:1024 + HALO],
                                                 src, AF.Identity, bias=bias)

        def rt1_copy(lo, hi, eng=None):
            # rt1[:, c, j] = rt[:, c, j+1]
            if eng is None:
                nc.scalar.copy(rt1_all[:, :, lo:hi], rt_all[:, 0:15, lo + 1:hi + 1])
            else:
                eng.tensor_copy(rt1_all[:, :, lo:hi], rt_all[:, 0:15, lo + 1:hi + 1])

        def norm128(w):
            """softmax denom + normalize et over cols [128w, 128w+128)."""
            lo = 128 * w
            e = et_all[:, :, lo:lo + 128]
            nc.vector.tensor_tensor(zts[:], e[:, 0:8, :], e[:, 8:16, :],
                                    op=mybir.AluOpType.add)
            nc.vector.tensor_tensor(zts[:, 0:4, :], zts[:, 0:4, :], zts[:, 4:8, :],
                                    op=mybir.AluOpType.add)
            nc.vector.tensor_tensor(zts[:, 0:2, :], zts[:, 0:2, :], zts[:, 2:4, :],
                                    op=mybir.AluOpType.add)
            nc.vector.tensor_tensor(zt[:], zts[:, 0, :], zts[:, 1, :],
                                    op=mybir.AluOpType.add)
            nc.vector.reciprocal(zt[:], zt[:])
            nc.vector.tensor_copy(rzb[:], zt[:])
            rz3 = rzb[:].rearrange("p (o n) -> p o n", o=1) \
                .to_broadcast((128, 15, 128))
            ew = et_all[:, 0:15, lo:lo + 128]
            nc.vector.tensor_mul(ew, ew, rz3)

        def mac(cp):
            """Banded MAC + writer for one 128-row chunklet pair."""
            mp = macp.tile([128, 16, 128], F32, tag="macp", name="mp")
            nc.vector.memset(mp[:, 0, :], 0.0)
            n0 = 128 * cp
            for a in range(15):
                cnt = 15 - a
                p = prodp.tile([128, 15, 128], BF16, tag="prodp", name="p")
                eb = et_all[:, a:a + 1, n0:n0 + 128].to_broadcast((128, cnt, 128))
                eng = nc.gpsimd if a in GP_A else nc.vector
                if a % 2 == 0:
                    # even shift a+1 is 2B-misaligned on rt; use the
                    # pre-shifted rt1 so DVE runs in 2x mode
                    eng.tensor_mul(p[:, 0:cnt, :], eb,
                                   rt1_all[:, 0:cnt, n0 + a:n0 + a + 128])
                else:
                    eng.tensor_mul(p[:, 0:cnt, :], eb,
                                   rt_all[:, 0:cnt, n0 + a + 1:n0 + a + 129])
                for b in range(4):
                    d_lo = max(a + 1, 4 * b)
                    d_hi = 4 * b + 4
                    if d_lo >= d_hi:
                        continue
                    last_a = min(14, 4 * b + 2)
                    nc.tensor.matmul(mp[:, d_lo:d_hi, :], ident[:],
                                     p[:, d_lo - a - 1:d_hi - a - 1, :],
                                     start=(a == 0), stop=(a == last_a))

            for g in range(2):
                cb = 8 * g + cp
                qt = qtp.tile([128, 8, 128], BF16, tag="qtp", name="qt")
                for dsub in range(2):
                    csrc = mp[64 * g:64 * g + 64, dsub::2, :]
                    cdst = qt[64 * dsub:64 * dsub + 64, :, :]
                    nc.scalar.copy(cdst, csrc)
                if cb == 15:
                    nc.vector.tensor_mul(qt[:], qt[:], qmask[:])
                osb = osbp.tile([128, 1024], F32, tag="osbp", name="osb")
                for h in range(2):
                    wp = wps.tile([128, 512], F32, tag="wps", name="wp")
                    for k in range(8):
                        nc.tensor.matmul(wp[:], qt[:, k, :],
                                         ww_sb[:, k, h * 512:(h + 1) * 512],
                                         start=(k == 0), stop=(k == 7))
                    nc.vector.tensor_copy(osb[:, h * 512:(h + 1) * 512], wp[:])
                nc.sync.dma_start(out_d[cb], osb[:])

        # ---------------- interleaved schedule
        reader(0)
        reader(2)
        for w in range(4):
            norm128(w)
        rt1_copy(0, 511, eng=nc.vector)
        reader(1)
        mac(0)
        mac(1)
        mac(2)
        # last supertile R-slabs first so rt (and rt1) unblock mac(3) early
        reader(3, uorder=list(range(8, 16)))
        rt1_copy(511, 655, eng=nc.vector)
        reader(3, uorder=list(range(0, 8)))
        reader(4)
        rt1_copy(655, 1023)
        rt1_copy(1023, RTC - 1)
        mac(3)
        norm128(4)
        mac(4)
        norm128(5)
        mac(5)
        norm128(6)
        mac(6)
        norm128(7)
        mac(7)

    nc.compile()
    return nc


def _host_prep(x, W_r, b_r, W_w, b_w):
    """Build the 8 per-core input maps."""
    xf = np.asarray(x, np.float32).reshape(B * N, PD)
    wr = np.asarray(W_r, np.float32).astype(bfnp)
    ww = np.asarray(W_w, np.float32).astype(bfnp)
    br = np.ascontiguousarray(
        np.asarray(b_r, np.float32).reshape(16, 128).T)
    # wr_d[u, p, ks*128+c] = W_r[128*ks + p, 128*u + c]
    wr_t = np.ascontiguousarray(
        wr.reshape(8, 128, 16, 128).transpose(2, 1, 0, 3).reshape(16, 128, 1024))
    ww_t = np.ascontiguousarray(ww.reshape(8, 128, 1024))
    ident = np.eye(128, dtype=np.float32).astype(bfnp)

    in_maps = []
    for c in range(8):
        lo = c * ROWS_PER_CORE
        chunk = np.zeros((NROWS, PD), np.float32)
        chunk[:ROWS_PER_CORE] = xf[lo:lo + ROWS_PER_CORE]
        if c % 2 == 0:
            chunk[ROWS_PER_CORE:] = xf[lo + ROWS_PER_CORE: lo + NROWS]
        # xT[ks, k, n] = chunk[n, 128*ks + k]
        xt = np.ascontiguousarray(
            chunk.astype(bfnp).reshape(NROWS, 8, 128).transpose(1, 2, 0))
        qmask = np.ones((128, 8, 128), np.float32)
        if c % 2 == 1:
            dsub = (np.arange(128)[:, None, None] // 64)
            k = np.arange(8)[None, :, None]
            n = np.arange(128)[None, None, :]
            bad = (n >= 112) & ((n - 112 + 2 * k + dsub) >= 16)
            qmask[np.broadcast_to(bad, (128, 8, 128))] = 0.0
        in_maps.append({
            "xT": xt,
            "wr": wr_t, "ww": ww_t, "br": br,
            "ident": ident, "qmask": qmask.astype(bfnp),
        })
    return in_maps


def kernel(x, W_r, b_r, W_w, b_w):
    if "nc" not in _cache:
        _cache["nc"] = _build()
    nc = _cache["nc"]
    in_maps = _host_prep(x, W_r, b_r, W_w, b_w)
    res = bass_utils.run_bass_kernel_spmd(nc, in_maps, core_ids=list(range(8)))
    out = np.concatenate([r["out"].reshape(ROWS_PER_CORE, ALL)
                          for r in res.results], axis=0)
    out = out.reshape(B, N, ALL).astype(np.float32)
    out += np.asarray(b_w, np.float32).reshape(1, 1, ALL)
    return np.ascontiguousarray(out)


# revision 19
# speedup vs baseline: 1.2679x; 1.0605x over previous
"""TRN2 Bass kernel for nn_ChartOperator (sparse_attention).

Math (B=4, N=4096, PD=1024, D=16, S=64, ALL=1024):
  P = x @ W_r + b_r
  L = P[..., :ALL].reshape(n, D, S); R = P[..., ALL:].reshape(n, D, S)
  w = softmax_a(L)
  Q[n, d, s] = sum_{a<d} w[n,a,s] * R[n+a+1, d-1-a, s]
  (last D rows of each batch: Q[t+d>=16] zeroed)
  out = Q.reshape(n, ALL) @ W_w + b_w

Sharding: 8 cores, data-parallel over flattened (B*N) rows, 2048 rows/core
plus a 16-row forward halo (zero-padded at batch boundaries; the affected
outputs are exactly the masked ones).

Per-core pipeline (bf16 matmuls, fp32 PSUM), software-pipelined so the
banded-MAC + writer chunklets interleave with the reader supertiles:
  PE order: rd(0) rd(2) rd(1) mac(0..2) rd(3,R-first) rd(halo) mac(3..7)
  1. Reader computes P TRANSPOSED: psum[(d2,s64), n512] = W_r_slab.T @ xT.
  2. ACT exp/copy writes banded-MAC layout:
     e chain  et_all[(g2,s64), a16, n1024]  (g: row-blocks 0-7 / 8-15)
     r chain  rt_all[(g2,s64), c16, n1040]  (blocks 0-8 / 8-16 incl halo)
     rt1_all = rt_all shifted by 1 col (so every banded product has a
     4B-aligned operand -> DVE 2x mode).
  3. softmax denom Z on GpSimd (strided reduce); DVE recip + normalize.
  4. banded products p[s, c, n] = w[s,a,n]*r[s,c,n+a+1]: even-a via rt1
     (DVE 2x), a in {0,2,4} offloaded to GpSimd.
  5. PE identity-matmuls accumulate products into PSUM Q[(g,s), d, n]
     with shrinking d-windows (d = a+c+1)
  6. ACT/DVE strided copies Q -> qt[(dsub2,s64), k8, n128] bf16
  7. writer matmuls out[n128, 1024] = qt.T @ W_w + b_w -> DMA out
"""
import numpy as np
import ml_dtypes
from contextlib import ExitStack

import concourse.bass as bass
import concourse.tile as tile
from concourse import bacc, mybir
from concourse import bass_utils

BF16 = mybir.dt.bfloat16
F32 = mybir.dt.float32
bfnp = ml_dtypes.bfloat16

B, N, PD = 4, 4096, 1024
D, S = 16, 64
ALL = D * S
ROWS_PER_CORE = 2048
HALO = 16
NROWS = ROWS_PER_CORE + HALO   # 2064
RTC = 1040                     # rt chain length (needs cols <= 1039)
NCP = 8
GP_A = (0, 2)                  # product shifts computed on GpSimd

_cache = {}


def _build(debug=False):
    nc = bacc.Bacc("TRN2", target_bir_lowering=False, debug=False, num_devices=8)

    xT_d = nc.dram_tensor("xT", [8, 128, NROWS], BF16, kind="ExternalInput").ap()
    wr_d = nc.dram_tensor("wr", [16, 128, 1024], BF16, kind="ExternalInput").ap()
    ww_d = nc.dram_tensor("ww", [8, 128, 1024], BF16, kind="ExternalInput").ap()
    br_d = nc.dram_tensor("br", [128, 16], F32, kind="ExternalInput").ap()
    ident_d = nc.dram_tensor("ident", [128, 128], BF16, kind="ExternalInput").ap()
    qmask_d = nc.dram_tensor("qmask", [128, 8, 128], BF16, kind="ExternalInput").ap()
    out_d = nc.dram_tensor("out", [16, 128, 1024], F32, kind="ExternalOutput").ap()

    AF = mybir.ActivationFunctionType

    with tile.TileContext(nc) as tc, ExitStack() as ctx:
        cpool = ctx.enter_context(tc.tile_pool(name="cpool", bufs=1))
        ps512 = ctx.enter_context(tc.tile_pool(name="ps512", bufs=2, space="PSUM"))
        wps = ctx.enter_context(tc.tile_pool(name="wps", bufs=2, space="PSUM"))
        macp = ctx.enter_context(tc.tile_pool(name="macp", bufs=1, space="PSUM"))
        prodp = ctx.enter_context(tc.tile_pool(name="prodp", bufs=3))
        qtp = ctx.enter_context(tc.tile_pool(name="qtp", bufs=4))
        osbp = ctx.enter_context(tc.tile_pool(name="osbp", bufs=3))

        # --- persistent tiles
        xkj = [cpool.tile([128, 8, 512], BF16, name=f"xkj{j}", tag=f"xkj{j}")
               for j in range(4)]
        xh = cpool.tile([128, 8, HALO], BF16, name="xh", tag="xh")
        wr_sb = cpool.tile([128, 16, 1024], BF16)   # (p, u, ks*128+c)
        ww_sb = cpool.tile([128, 8, 1024], BF16)
        br_sb = cpool.tile([128, 16], F32)
        ident = cpool.tile([128, 128], BF16)
        qmask = cpool.tile([128, 8, 128], BF16)
        et_all = cpool.tile([128, 16, 1024], BF16)   # [(g2,s64), a, n-chain]
        rt_all = cpool.tile([128, 16, RTC], BF16)    # [(g2,s64), c, n-chain]
        rt1_all = cpool.tile([128, 15, RTC], BF16)   # rt shifted by +1 col
        zts = cpool.tile([128, 4, 128], F32)         # denom tree scratch
        zt = cpool.tile([128, 128], F32)             # softmax denom
        rzb = cpool.tile([128, 128], BF16)           # 1/Z bf16

        # --- input DMAs on sync, strictly ordered by first use so the
        # first-supertile weights/activations get the full DMA bandwidth
        nc.sync.dma_start(xkj[0][:], xT_d[:, :, 0:512].rearrange("k p n -> p k n"))
        nc.sync.dma_start(wr_sb[:, 0, :], wr_d[0])
        nc.sync.dma_start(wr_sb[:, 1:4, :], wr_d[1:4].rearrange("u p c -> p u c"))
        nc.sync.dma_start(br_sb[:], br_d[:])
        nc.sync.dma_start(wr_sb[:, 4:8, :], wr_d[4:8].rearrange("u p c -> p u c"))
        nc.sync.dma_start(wr_sb[:, 8:12, :], wr_d[8:12].rearrange("u p c -> p u c"))
        nc.sync.dma_start(wr_sb[:, 12:16, :], wr_d[12:16].rearrange("u p c -> p u c"))
        nc.sync.dma_start(xkj[2][:], xT_d[:, :, 1024:1536].rearrange("k p n -> p k n"))
        nc.sync.dma_start(xkj[1][:], xT_d[:, :, 512:1024].rearrange("k p n -> p k n"))
        nc.sync.dma_start(ident[:], ident_d[:])
        nc.sync.dma_start(xkj[3][:], xT_d[:, :, 1536:2048].rearrange("k p n -> p k n"))
        nc.sync.dma_start(xh[:], xT_d[:, :, 2048:NROWS].rearrange("k p n -> p k n"))
        nc.sync.dma_start(ww_sb[:, 0:4, :], ww_d[0:4].rearrange("k p c -> p k c"))
        nc.sync.dma_start(ww_sb[:, 4:8, :], ww_d[4:8].rearrange("k p c -> p k c"))
        nc.sync.dma_start(qmask[:], qmask_d[:])

        def reader(jj, uorder=None):
            """One 512-row supertile (jj<4) or the 16-row halo (jj==4)."""
            halo = jj == 4
            nwin = HALO if halo else 512
            n0 = jj * 512
            g = 0 if jj < 2 else 1
            xt = xh if halo else xkj[jj]
            for u in (uorder or range(16)):
                is_l = u < 8
                if is_l and halo:
                    continue                  # halo rows: R only
                ps = ps512.tile([128, 512], F32, tag="ps512", name="ps")
                for ks in range(8):
                    nc.tensor.matmul(ps[:, :nwin], wr_sb[:, u, 128 * ks:128 * ks + 128],
                                     xt[:, ks, :nwin],
                                     start=(ks == 0), stop=(ks == 7))
                for dsub in range(2):
                    src = ps[64 * dsub:64 * dsub + 64, :nwin]
                    bias = br_sb[64 * dsub:64 * dsub + 64, u:u + 1]
                    if is_l:
                        a = 2 * u + dsub
                        dst = et_all[64 * g:64 * g + 64, a,
                                     n0 - 1024 * g:n0 - 1024 * g + nwin]
                        nc.scalar.activation(dst, src, AF.Exp, bias=bias)
                    else:
                        c = 2 * (u - 8) + dsub
                        # g0 chain: blocks 0..8+ ; g1 chain: blocks 8..16+
                        if jj < 2:
                            nc.scalar.activation(rt_all[0:64, c, n0:n0 + 512], src,
                                                 AF.Identity, bias=bias)
                        elif jj == 2:   # blocks 8-11: both chains
                            nc.scalar.activation(rt_all[0:64, c, 1024:RTC],
                                                 ps[64 * dsub:64 * dsub + 64, 0:RTC - 1024],
                                                 AF.Identity, bias=bias)
                            nc.scalar.activation(rt_all[64:128, c, 0:512], src,
                                                 AF.Identity, bias=bias)
                        elif jj == 3:
                            nc.scalar.activation(rt_all[64:128, c, 512:1024], src,
                                                 AF.Identity, bias=bias)
                        else:           # halo block 16
                            nc.scalar.activation(rt_all[64:128, c, 1024:1024 + HALO],
                                                 src, AF.Identity, bias=bias)

        def rt1_copy(lo, hi, eng=None):
            # rt1[:, c, j] = rt[:, c, j+1]
            if eng is None:
                nc.scalar.copy(rt1_all[:, :, lo:hi], rt_all[:, 0:15, lo + 1:hi + 1])
            else:
                eng.tensor_copy(rt1_all[:, :, lo:hi], rt_all[:, 0:15, lo + 1:hi + 1])

        def norm128(w):
            """softmax denom + normalize et over cols [128w, 128w+128)."""
            lo = 128 * w
            e = et_all[:, :, lo:lo + 128]
            nc.vector.tensor_tensor(zts[:], e[:, 0:4, :], e[:, 4:8, :],
                                    op=mybir.AluOpType.add)
            nc.vector.tensor_tensor(zts[:], zts[:], e[:, 8:12, :],
                                    op=mybir.AluOpType.add)
            nc.vector.tensor_tensor(zts[:], zts[:], e[:, 12:16, :],
                                    op=mybir.AluOpType.add)
            nc.vector.tensor_tensor(zts[:, 0:2, :], zts[:, 0:2, :], zts[:, 2:4, :],
                                    op=mybir.AluOpType.add)
            nc.vector.tensor_tensor(zt[:], zts[:, 0, :], zts[:, 1, :],
                                    op=mybir.AluOpType.add)
            nc.vector.reciprocal(zt[:], zt[:])
            nc.vector.tensor_copy(rzb[:], zt[:])
            rz3 = rzb[:].rearrange("p (o n) -> p o n", o=1) \
                .to_broadcast((128, 15, 128))
            ew = et_all[:, 0:15, lo:lo + 128]
            nc.vector.tensor_mul(ew, ew, rz3)

        qts = {}

        def band(cp):
            """Banded MAC for one 128-row chunklet pair -> qt tiles."""
            mp = macp.tile([128, 16, 128], F32, tag="macp", name="mp")
            n0 = 128 * cp
            for a in range(15):
                cnt = 15 - a
                p = prodp.tile([128, 15, 128], BF16, tag="prodp", name="p")
                eb = et_all[:, a:a + 1, n0:n0 + 128].to_broadcast((128, cnt, 128))
                eng = nc.gpsimd if a in GP_A else nc.vector
                if a % 2 == 0:
                    # even shift a+1 is 2B-misaligned on rt; use the
                    # pre-shifted rt1 so DVE runs in 2x mode
                    eng.tensor_mul(p[:, 0:cnt, :], eb,
                                   rt1_all[:, 0:cnt, n0 + a:n0 + a + 128])
                else:
                    eng.tensor_mul(p[:, 0:cnt, :], eb,
                                   rt_all[:, 0:cnt, n0 + a + 1:n0 + a + 129])
                for b in range(4):
                    d_lo = max(a + 1, 4 * b)
                    d_hi = 4 * b + 4
                    if d_lo >= d_hi:
                        continue
                    last_a = min(14, 4 * b + 2)
                    nc.tensor.matmul(mp[:, d_lo:d_hi, :], ident[:],
                                     p[:, d_lo - a - 1:d_hi - a - 1, :],
                                     start=(a == 0), stop=(a == last_a))
            nc.vector.memset(mp[:, 0, :], 0.0)

            for g in range(2):
                qt = qtp.tile([128, 8, 128], BF16, tag="qtp", name="qt")
                for dsub in range(2):
                    csrc = mp[64 * g:64 * g + 64, dsub::2, :]
                    cdst = qt[64 * dsub:64 * dsub + 64, :, :]
                    nc.scalar.copy(cdst, csrc)
                if 8 * g + cp == 15:
                    nc.vector.tensor_mul(qt[:], qt[:], qmask[:])
                qts[(cp, g)] = qt

        def writer(cp):
            """Writer matmuls for chunklet cp (runs one cp behind band)."""
            for g in range(2):
                cb = 8 * g + cp
                qt = qts.pop((cp, g))
                for h in range(2):
                    wp = wps.tile([128, 512], F32, tag="wps", name="wp")
                    for k in range(8):
                        nc.tensor.matmul(wp[:], qt[:, k, :],
                                         ww_sb[:, k, h * 512:(h + 1) * 512],
                                         start=(k == 0), stop=(k == 7))
                    osb = osbp.tile([128, 512], F32, tag="osbp", name="osb")
                    nc.scalar.copy(osb[:], wp[:])
                    nc.sync.dma_start(out_d[cb][:, h * 512:(h + 1) * 512], osb[:])

        # ---------------- interleaved schedule
        reader(0)
        reader(2)
        for w in range(4):
            norm128(w)
        rt1_copy(0, 511, eng=nc.vector)
        reader(1)
        band(0)
        band(1)
        writer(0)
        band(2)
        writer(1)
        # last supertile R-slabs first so rt (and rt1) unblock band(3) early
        reader(3, uorder=list(range(8, 16)))
        rt1_copy(511, 655, eng=nc.vector)
        rt1_copy(655, 1023, eng=nc.vector)
        reader(3, uorder=list(range(0, 8)))
        reader(4)
        band(3)
        writer(2)
        norm128(4)
        rt1_copy(1023, RTC - 1, eng=nc.vector)
        band(4)
        writer(3)
        norm128(5)
        band(5)
        writer(4)
        norm128(6)
        band(6)
        writer(5)
        norm128(7)
        band(7)
        writer(6)
        writer(7)

    nc.compile()
    return nc


def _host_prep(x, W_r, b_r, W_w, b_w):
    """Build the 8 per-core input maps."""
    xf = np.asarray(x, np.float32).reshape(B * N, PD)
    wr = np.asarray(W_r, np.float32).astype(bfnp)
    ww = np.asarray(W_w, np.float32).astype(bfnp)
    br = np.ascontiguousarray(
        np.asarray(b_r, np.float32).reshape(16, 128).T)
    # wr_d[u, p, ks*128+c] = W_r[128*ks + p, 128*u + c]
    wr_t = np.ascontiguousarray(
        wr.reshape(8, 128, 16, 128).transpose(2, 1, 0, 3).reshape(16, 128, 1024))
    ww_t = np.ascontiguousarray(ww.reshape(8, 128, 1024))
    ident = np.eye(128, dtype=np.float32).astype(bfnp)

    in_maps = []
    for c in range(8):
        lo = c * ROWS_PER_CORE
        chunk = np.zeros((NROWS, PD), np.float32)
        chunk[:ROWS_PER_CORE] = xf[lo:lo + ROWS_PER_CORE]
        if c % 2 == 0:
            chunk[ROWS_PER_CORE:] = xf[lo + ROWS_PER_CORE: lo + NROWS]
        # xT[ks, k, n] = chunk[n, 128*ks + k]
        xt = np.ascontiguousarray(
            chunk.astype(bfnp).reshape(NROWS, 8, 128).transpose(1, 2, 0))
        qmask = np.ones((128, 8, 128), np.float32)
        if c % 2 == 1:
            dsub = (np.arange(128)[:, None, None] // 64)
            k = np.arange(8)[None, :, None]
            n = np.arange(128)[None, None, :]
            bad = (n >= 112) & ((n - 112 + 2 * k + dsub) >= 16)
            qmask[np.broadcast_to(bad, (128, 8, 128))] = 0.0
        in_maps.append({
            "xT": xt,
            "wr": wr_t, "ww": ww_t, "br": br,
            "ident": ident, "qmask": qmask.astype(bfnp),
        })
    return in_maps


def kernel(x, W_r, b_r, W_w, b_w):
    if "nc" not in _cache:
        _cache["nc"] = _build()
    nc = _cache["nc"]
    in_maps = _host_prep(x, W_r, b_r, W_w, b_w)
    res = bass_utils.run_bass_kernel_spmd(nc, in_maps, core_ids=list(range(8)))
    out = np.concatenate([r["out"].reshape(ROWS_PER_CORE, ALL)
                          for r in res.results], axis=0)
    out = out.reshape(B, N, ALL).astype(np.float32)
    out += np.asarray(b_w, np.float32).reshape(1, 1, ALL)
    return np.ascontiguousarray(out)
